# revision 16
# baseline (speedup 1.0000x reference)
"""Multi-head causal self-attention with RoPE on 8 Trainium2 NeuronCores.

Reference computation (B=2, S=2048, D=2048, H=16, DH=128):
    xs = hidden_q / sqrt(D)
    q,k,v = xs @ {Wq,Wk,Wv}.T        (reshaped to [B,H,S,DH])
    q,k <- RoPE(q,k)
    scores = q @ k.T / sqrt(DH)  (causal masked)
    p = softmax(scores); attn = p @ v
    out = (attn / sqrt(H*DH)) @ Wo.T

Sharding: 8 cores = 2 (batch) x 4 (head-groups of 4 heads).  Each core
computes its head-group's projections, attention and a partial output
projection; the host sums the 4 partials per batch.

v6 design (all matmul operands fp16, PSUM fp32):
  * Q^T/K^T produced directly in [dh, seq] layout (weights stationary,
    x^T moving): no PE transposes, no DRAM spills.  RoPE uses a signed
    sin table (rows 0-63 hold -sin): 4 cross-partition DVE ops.
  * Causal mask added to scores in PSUM via identity.T @ (-30000 band);
    exp underflows to exact 0 -- the DVE stays out of the softmax chain.
  * Softmax denominator accumulates in a [1,512] PSUM bank via a
    ones-vector matmul per key tile (PE slots are cheaper than DVE ops
    here: DVE [128,512] ~800ns vs PE N=512 ~216ns).
  * Attention/denominator matmuls trail the score/exp pipeline by TWO
    key tiles so no PE instruction waits on a fresh exp semaphore
    (queue-head waits block the LDWEIGHTS pull-ahead, costing ~95ns on
    each following matmul).
  * attn output overwrites qT in place (the q slice of a (h,qb) is dead
    once its scores are done).
  * V-projection copies go through the Vector engine and y staging
    through Scalar, keeping the ACT queue clear of work that could
    delay phase-B exps.
  * DMA: wq/x0 split per contraction tile (gpsimd/sync queues), wk on
    the scalar queue, so the first projection chains chase arrivals.
    y partials are fp16; host sums 4 partials per batch in fp32.
"""

import math
from contextlib import ExitStack

import numpy as np

import concourse.bass as bass
import concourse.mybir as mybir
import concourse.tile as tile
from concourse import bacc
from concourse.bass import ts
from concourse.bass_utils import run_bass_kernel_spmd
from concourse.masks import make_identity

B, S, D, H, DH = 2, 2048, 2048, 16, 128
BASE = 10000.0
G = 4              # head-groups (cores per batch)
HG = H // G        # heads per group = 4
F = HG * DH        # features per group = 512
NT = S // 128      # 16 token tiles
NKT = D // 128     # 16 contraction tiles
NQB = S // 512     # 4 query blocks
NEG = -30000.0     # causal-mask bias; exp((s+NEG)/sqrt(DH)) == 0
F32 = mybir.dt.float32
F16 = mybir.dt.float16

_cache = {}


def _rope_tables():
    # [dh=128, S]; cos duplicated halves; sin rows 0-63 carry -sin
    inv_freq = 1.0 / (BASE ** (np.arange(0, DH, 2, dtype=np.float64) / DH))
    t = np.arange(S, dtype=np.float64)
    freqs = np.outer(inv_freq, t)                       # [64, S]
    cosT = np.concatenate([np.cos(freqs), np.cos(freqs)], 0)
    sinT = np.concatenate([-np.sin(freqs), np.sin(freqs)], 0)
    return cosT.astype(np.float16), sinT.astype(np.float16)


def _mask_tiles():
    # negmask[o][j, q] = 0 where key j+128*o <= query q, else NEG
    o = np.arange(4)[:, None, None]
    j = np.arange(128)[None, :, None]
    q = np.arange(512)[None, None, :]
    return np.where(q >= j + 128 * o, 0.0, NEG).astype(np.float16)


def _build(reps=1):
    key = ("nc", reps)
    if key in _cache:
        return _cache[key]
    nc = bacc.Bacc("TRN2", target_bir_lowering=False, debug=False, num_devices=8)

    xT = nc.dram_tensor("xT", [D, S], F16, kind="ExternalInput")
    wqT = nc.dram_tensor("wqT", [D, F], F16, kind="ExternalInput")
    wkT = nc.dram_tensor("wkT", [D, F], F16, kind="ExternalInput")
    wvT = nc.dram_tensor("wvT", [D, F], F16, kind="ExternalInput")
    woT = nc.dram_tensor("woT", [F, D], F16, kind="ExternalInput")
    cos_d = nc.dram_tensor("cos", [128, S], F16, kind="ExternalInput")
    sin_d = nc.dram_tensor("sin", [128, S], F16, kind="ExternalInput")
    msk_d = nc.dram_tensor("masks", [4, 128, 512], F16, kind="ExternalInput")
    y = nc.dram_tensor("y", [S, D], F16, kind="ExternalOutput")

    xT_r = xT.ap().rearrange("(kt p) s -> p kt s", p=128)       # [128, 16, S]
    wqT_r = wqT.ap().rearrange("(kt p) f -> p kt f", p=128)
    wkT_r = wkT.ap().rearrange("(kt p) f -> p kt f", p=128)
    wvT_r = wvT.ap().rearrange("(kt p) f -> p kt f", p=128)
    woT_r = woT.ap().rearrange("(ft p) d -> p ft d", p=128)

    with tile.TileContext(nc) as tc, ExitStack() as ctx:
        const = ctx.enter_context(tc.tile_pool(name="const", bufs=1))
        wpool = ctx.enter_context(tc.tile_pool(name="wpool", bufs=1))
        xpool = ctx.enter_context(tc.tile_pool(name="xpool", bufs=2))
        big = ctx.enter_context(tc.tile_pool(name="big", bufs=1))
        pt_pool = ctx.enter_context(tc.tile_pool(name="pt", bufs=6))
        tmp_pool = ctx.enter_context(tc.tile_pool(name="tmp", bufs=2))
        nrm = ctx.enter_context(tc.tile_pool(name="nrm", bufs=2))
        ystage = ctx.enter_context(tc.tile_pool(name="ystage", bufs=4))
        # PSUM: 2 + 3 + 2 + 1 banks = 8
        psA = ctx.enter_context(tc.tile_pool(name="psA", bufs=2, space="PSUM"))
        psS = ctx.enter_context(tc.tile_pool(name="psS", bufs=3, space="PSUM"))
        psT = ctx.enter_context(tc.tile_pool(name="psT", bufs=2, space="PSUM"))
        psD = ctx.enter_context(tc.tile_pool(name="psD", bufs=1, space="PSUM"))

        ones = const.tile([128, 1], F16, tag="ones")
        nc.gpsimd.memset(ones[:], 1.0)
        ident = const.tile([128, 128], F16, tag="ident")
        make_identity(nc, ident[:])
        msk_sb = const.tile([128, 4, 512], F16, tag="masks")
        nc.scalar.dma_start(msk_sb[:], msk_d.ap().rearrange("o p q -> p o q"))

        # static loads; first chains chase per-kt arrivals
        wq_sb = wpool.tile([128, NKT, F], F16, tag="wq")
        wk_sb = wpool.tile([128, NKT, F], F16, tag="wk")
        wv_sb = wpool.tile([128, NKT, F], F16, tag="wv")
        wo_sb = wpool.tile([128, G, D], F16, tag="wo")
        cos_sb = wpool.tile([128, S], F16, tag="cos")
        sin_sb = wpool.tile([128, S], F16, tag="sin")
        for kt in range(NKT):
            nc.gpsimd.dma_start(wq_sb[:, kt, :], wqT_r[:, kt, :])
        nc.scalar.dma_start(cos_sb[:], cos_d.ap())
        nc.scalar.dma_start(sin_sb[:], sin_d.ap())
        for kt in range(NKT):
            nc.scalar.dma_start(wk_sb[:, kt, :], wkT_r[:, kt, :])
        for kt in range(NKT):
            # split wv across both weight queues so phase-A V chains
            # are fed by the time Q/K chains finish
            eng = nc.gpsimd if kt % 2 == 0 else nc.scalar
            eng.dma_start(wv_sb[:, kt, :], wvT_r[:, kt, :])
        nc.scalar.dma_start(wo_sb[:], woT_r)

        for _rep in range(reps):
            qT = big.tile([128, HG, S], F16, tag="qT", name="qT")
            kT = big.tile([128, HG, S], F16, tag="kT", name="kT")
            v_sb = big.tile([128, NT, F], F16, tag="v", name="v")
            attn_sb = big.tile([128, HG, S], F16, tag="attn", name="attn")

            x_blocks = {}
            for sb in range(2):
                x_blocks[sb] = xpool.tile([128, NKT, 512], F16, tag="x",
                                          name=f"x{sb}")
                if sb == 0:
                    for kt in range(NKT):
                        nc.sync.dma_start(x_blocks[0][:, kt, :],
                                          xT_r[:, kt, ts(0, 512)])
                else:
                    nc.sync.dma_start(x_blocks[sb][:],
                                      xT_r[:, :, ts(sb, 512)])

            for sb in range(NQB):
                # ---------------- Phase A: projections + RoPE --------------
                x_sb = x_blocks.pop(sb)
                if sb + 2 < NQB:
                    x_blocks[sb + 2] = xpool.tile([128, NKT, 512], F16,
                                                  tag="x", name=f"x{sb+2}")
                    nc.sync.dma_start(x_blocks[sb + 2][:],
                                      xT_r[:, :, ts(sb + 2, 512)])
                sbs = ts(sb, 512)
                for h in range(HG):
                    for (w_sb, out_t) in ((wq_sb, qT), (wk_sb, kT)):
                        ps = psA.tile([128, 512], F32, tag="psA")
                        for kt in range(NKT):
                            nc.tensor.matmul(ps[:], w_sb[:, kt, ts(h, 128)],
                                             x_sb[:, kt, :],
                                             start=(kt == 0),
                                             stop=(kt == NKT - 1))
                        # RoPE: out = ps*cos + rot_half(ps)*sin
                        tmp = tmp_pool.tile([128, 512], F16, tag="rtmp")
                        nc.vector.tensor_mul(tmp[0:64, :], ps[64:128, :],
                                             sin_sb[0:64, sbs])
                        nc.vector.tensor_mul(tmp[64:128, :], ps[0:64, :],
                                             sin_sb[64:128, sbs])
                        dst = out_t[:, h, sbs]
                        nc.vector.tensor_mul(dst, ps[:], cos_sb[:, sbs])
                        nc.vector.tensor_add(dst, dst, tmp[:])
                for st in range(4):
                    ps = psA.tile([128, 512], F32, tag="psA")
                    for kt in range(NKT):
                        nc.tensor.matmul(ps[:], x_sb[:, kt, ts(st, 128)],
                                         wv_sb[:, kt, :],
                                         start=(kt == 0),
                                         stop=(kt == NKT - 1))
                    nc.vector.tensor_copy(v_sb[:, 4 * sb + st, :], ps[:])

                # ---------------- Phase B: attention for q-block sb --------
                qb = sb
                nkt = 4 * qb + 4
                for h in range(HG):
                    p_att = psT.tile([128, 512], F32, tag="psT")
                    p_den = psD.tile([1, 512], F32, tag="psD")
                    pts = {}

                    def drain(kt, last):
                        pt = pts.pop(kt)
                        nc.tensor.matmul(p_att[:], v_sb[:, kt, ts(h, 128)],
                                         pt[:],
                                         start=(kt == 0), stop=last)
                        nc.tensor.matmul(p_den[:], ones[:], pt[:],
                                         start=(kt == 0), stop=last)

                    for kt in range(nkt):
                        p_s = psS.tile([128, 512], F32, tag="psS")
                        diag = kt >= 4 * qb
                        nc.tensor.matmul(p_s[:], kT[:, h, ts(kt, 128)],
                                         qT[:, h, ts(qb, 512)],
                                         start=True, stop=not diag)
                        if diag:
                            # scores += I.T @ negmask (exp -> exact 0)
                            nc.tensor.matmul(p_s[:], ident[:],
                                             msk_sb[:, kt - 4 * qb, :],
                                             start=False, stop=True)
                        if kt >= 2:
                            drain(kt - 2, last=False)
                        pt = pt_pool.tile([128, 512], F16, tag="pt")
                        nc.scalar.activation(pt[:], p_s[:],
                                             mybir.ActivationFunctionType.Exp,
                                             scale=1.0 / math.sqrt(DH))
                        pts[kt] = pt
                    drain(nkt - 2, last=False)
                    drain(nkt - 1, last=True)
                    recip = nrm.tile([1, 512], F32, tag="recip")
                    nc.vector.reciprocal_approx_fast(recip[:], p_den[:])
                    rb = nrm.tile([128, 512], F32, tag="rb")
                    nc.gpsimd.partition_broadcast(rb[:], recip[:])
                    nc.vector.tensor_mul(attn_sb[:, h, ts(qb, 512)],
                                         p_att[:], rb[:])

                # ---------------- Phase C: output projection ---------------
                for qt in range(4 * qb, 4 * qb + 4):
                    for db in range(NQB):
                        py = psA.tile([128, 512], F32, tag="psA")
                        for ft in range(G):
                            nc.tensor.matmul(py[:],
                                             attn_sb[:, ft, ts(qt, 128)],
                                             wo_sb[:, ft, ts(db, 512)],
                                             start=(ft == 0),
                                             stop=(ft == G - 1))
                        y_sb = ystage.tile([128, 512], F16, tag="ysb")
                        nc.scalar.copy(y_sb[:], py[:])
                        nc.sync.dma_start(y.ap()[ts(qt, 128), ts(db, 512)],
                                          y_sb[:])

    nc.compile()
    _cache[key] = nc
    return nc


def _in_maps(hidden_q, Wq, Wk, Wv, Wo):
    xs = hidden_q.astype(np.float32) / math.sqrt(D)
    xT = [np.ascontiguousarray(xs[b].T).astype(np.float16) for b in range(B)]
    cos_t, sin_t = _rope_tables()
    masks = _mask_tiles()
    wo_s = Wo.astype(np.float32) / math.sqrt(H * DH)
    in_maps = []
    for c in range(8):
        b, g = c // G, c % G
        rows = slice(F * g, F * (g + 1))
        in_maps.append({
            "xT": xT[b],
            "wqT": np.ascontiguousarray(Wq[rows, :].T).astype(np.float16),
            "wkT": np.ascontiguousarray(Wk[rows, :].T).astype(np.float16),
            "wvT": np.ascontiguousarray(Wv[rows, :].T).astype(np.float16),
            "woT": np.ascontiguousarray(wo_s[:, rows].T).astype(np.float16),
            "cos": cos_t, "sin": sin_t, "masks": masks,
        })
    return in_maps


def kernel(hidden_q, attention_mask, position_bias, Wq, Wk, Wv, Wo):
    hidden_q = np.asarray(hidden_q)
    Wq, Wk, Wv, Wo = (np.asarray(w) for w in (Wq, Wk, Wv, Wo))
    assert hidden_q.shape == (B, S, D)
    in_maps = _in_maps(hidden_q, Wq, Wk, Wv, Wo)
    nc = _build()
    res = run_bass_kernel_spmd(nc, in_maps, core_ids=list(range(8)))
    _cache["last_results"] = res
    out = np.zeros((B, S, D), np.float32)
    for c in range(8):
        out[c // G] += res.results[c]["y"]
    return out


# revision 22
# speedup vs baseline: 1.1748x; 1.1748x over previous
"""Multi-head causal self-attention with RoPE on 8 Trainium2 NeuronCores.

Reference computation (B=2, S=2048, D=2048, H=16, DH=128):
    xs = hidden_q / sqrt(D)
    q,k,v = xs @ {Wq,Wk,Wv}.T        (reshaped to [B,H,S,DH])
    q,k <- RoPE(q,k)
    scores = q @ k.T / sqrt(DH)  (causal masked)
    p = softmax(scores); attn = p @ v
    out = (attn / sqrt(H*DH)) @ Wo.T

Sharding: 8 cores = 2 (batch) x 4 (head-groups of 4 heads).  Each core
computes its head-group's projections, attention and a partial output
projection; the host sums the 4 partials per batch.

v6 design (all matmul operands fp16, PSUM fp32):
  * Q^T/K^T produced directly in [dh, seq] layout (weights stationary,
    x^T moving): no PE transposes, no DRAM spills.  RoPE uses a signed
    sin table (rows 0-63 hold -sin): 4 cross-partition DVE ops.
  * Causal mask added to scores in PSUM via identity.T @ (-30000 band);
    exp underflows to exact 0 -- the DVE stays out of the softmax chain.
  * Softmax denominator accumulates in a [1,512] PSUM bank via a
    ones-vector matmul per key tile (PE slots are cheaper than DVE ops
    here: DVE [128,512] ~800ns vs PE N=512 ~216ns).
  * Attention/denominator matmuls trail the score/exp pipeline by TWO
    key tiles so no PE instruction waits on a fresh exp semaphore
    (queue-head waits block the LDWEIGHTS pull-ahead, costing ~95ns on
    each following matmul).
  * attn output overwrites qT in place (the q slice of a (h,qb) is dead
    once its scores are done).
  * V-projection copies go through the Vector engine and y staging
    through Scalar, keeping the ACT queue clear of work that could
    delay phase-B exps.
  * DMA: wq/x0 split per contraction tile (gpsimd/sync queues), wk on
    the scalar queue, so the first projection chains chase arrivals.
    y partials are fp16; host sums 4 partials per batch in fp32.
"""

import math
from contextlib import ExitStack

import numpy as np

import concourse.bass as bass
import concourse.mybir as mybir
import concourse.tile as tile
from concourse import bacc
from concourse.bass import ts
from concourse.bass_utils import run_bass_kernel_spmd
from concourse.masks import make_identity

B, S, D, H, DH = 2, 2048, 2048, 16, 128
BASE = 10000.0
G = 4              # head-groups (cores per batch)
HG = H // G        # heads per group = 4
F = HG * DH        # features per group = 512
NT = S // 128      # 16 token tiles
NKT = D // 128     # 16 contraction tiles
NQB = S // 512     # 4 query blocks
NEG = -30000.0     # causal-mask bias; exp((s+NEG)/sqrt(DH)) == 0
F32 = mybir.dt.float32
F16 = mybir.dt.float16

_cache = {}


def _rope_tables():
    # [dh=128, S]; cos duplicated halves; sin rows 0-63 carry -sin
    inv_freq = 1.0 / (BASE ** (np.arange(0, DH, 2, dtype=np.float64) / DH))
    t = np.arange(S, dtype=np.float64)
    freqs = np.outer(inv_freq, t)                       # [64, S]
    cosT = np.concatenate([np.cos(freqs), np.cos(freqs)], 0)
    sinT = np.concatenate([-np.sin(freqs), np.sin(freqs)], 0)
    return cosT.astype(np.float16), sinT.astype(np.float16)


def _mask_tiles():
    # negmask[o][j, q] = 0 where key j+128*o <= query q, else NEG
    o = np.arange(4)[:, None, None]
    j = np.arange(128)[None, :, None]
    q = np.arange(512)[None, None, :]
    return np.where(q >= j + 128 * o, 0.0, NEG).astype(np.float16)


def _build(reps=1):
    key = ("nc", reps)
    if key in _cache:
        return _cache[key]
    nc = bacc.Bacc("TRN2", target_bir_lowering=False, debug=False, num_devices=8)

    xT = nc.dram_tensor("xT", [D, S], F16, kind="ExternalInput")
    wqT = nc.dram_tensor("wqT", [D, F], F16, kind="ExternalInput")
    wkT = nc.dram_tensor("wkT", [D, F], F16, kind="ExternalInput")
    wvT = nc.dram_tensor("wvT", [D, F], F16, kind="ExternalInput")
    woT = nc.dram_tensor("woT", [F, D], F16, kind="ExternalInput")
    cos_d = nc.dram_tensor("cos", [128, S], F16, kind="ExternalInput")
    sin_d = nc.dram_tensor("sin", [128, S], F16, kind="ExternalInput")
    msk_d = nc.dram_tensor("masks", [4, 128, 512], F16, kind="ExternalInput")
    y = nc.dram_tensor("y", [S, D], F16, kind="ExternalOutput")

    xT_r = xT.ap().rearrange("(kt p) s -> p kt s", p=128)       # [128, 16, S]
    wqT_r = wqT.ap().rearrange("(kt p) f -> p kt f", p=128)
    wkT_r = wkT.ap().rearrange("(kt p) f -> p kt f", p=128)
    wvT_r = wvT.ap().rearrange("(kt p) f -> p kt f", p=128)
    woT_r = woT.ap().rearrange("(ft p) d -> p ft d", p=128)

    with tile.TileContext(nc) as tc, ExitStack() as ctx:
        const = ctx.enter_context(tc.tile_pool(name="const", bufs=1))
        wpool = ctx.enter_context(tc.tile_pool(name="wpool", bufs=1))
        xpool = ctx.enter_context(tc.tile_pool(name="xpool", bufs=2))
        big = ctx.enter_context(tc.tile_pool(name="big", bufs=1))
        pt_pool = ctx.enter_context(tc.tile_pool(name="pt", bufs=6))
        tmp_pool = ctx.enter_context(tc.tile_pool(name="tmp", bufs=2))
        nrm = ctx.enter_context(tc.tile_pool(name="nrm", bufs=2))
        ystage = ctx.enter_context(tc.tile_pool(name="ystage", bufs=4))
        # PSUM: 2 + 3 + 2 + 1 banks = 8
        psA = ctx.enter_context(tc.tile_pool(name="psA", bufs=2, space="PSUM"))
        psS = ctx.enter_context(tc.tile_pool(name="psS", bufs=3, space="PSUM"))
        psT = ctx.enter_context(tc.tile_pool(name="psT", bufs=2, space="PSUM"))
        psD = ctx.enter_context(tc.tile_pool(name="psD", bufs=1, space="PSUM"))

        ones = const.tile([128, 1], F16, tag="ones")
        nc.gpsimd.memset(ones[:], 1.0)
        ident = const.tile([128, 128], F16, tag="ident")
        make_identity(nc, ident[:])
        msk_sb = const.tile([128, 4, 512], F16, tag="masks")

        # static loads; first chains chase per-kt arrivals
        wq_sb = wpool.tile([128, NKT, F], F16, tag="wq")
        wk_sb = wpool.tile([128, NKT, F], F16, tag="wk")
        wv_sb = wpool.tile([128, NKT, F], F16, tag="wv")
        wo_sb = wpool.tile([128, G, D], F16, tag="wo")
        cos_sb = wpool.tile([128, S], F16, tag="cos")
        sin_sb = wpool.tile([128, S], F16, tag="sin")
        for _rep in range(reps):
            qT = big.tile([128, HG, S], F16, tag="qT", name="qT")
            kT = big.tile([128, HG, S], F16, tag="kT", name="kT")
            v_sb = big.tile([128, NT, F], F16, tag="v", name="v")
            attn_sb = big.tile([128, HG, S], F16, tag="attn", name="attn")

            x_blocks = {}
            for sb in range(2):
                x_blocks[sb] = xpool.tile([128, NKT, 512], F16, tag="x",
                                          name=f"x{sb}")

            # First-needed slices up front, then the critical 6MB
            # (wq/wk/x-block-0) round-robined across all three DMA queues so
            # the first Q/K chains chase tile arrivals at aggregate
            # bandwidth; late-needed tensors (wv, tables, masks, wo) after.
            nc.scalar.dma_start(cos_sb[:, ts(0, 512)],
                                cos_d.ap()[:, ts(0, 512)])
            nc.scalar.dma_start(sin_sb[:, ts(0, 512)],
                                sin_d.ap()[:, ts(0, 512)])
            queues = (nc.gpsimd, nc.sync, nc.scalar)
            for kt in range(NKT):
                queues[kt % 3].dma_start(wq_sb[:, kt, :], wqT_r[:, kt, :])
                queues[(kt + 1) % 3].dma_start(x_blocks[0][:, kt, :],
                                               xT_r[:, kt, ts(0, 512)])
                queues[(kt + 2) % 3].dma_start(wk_sb[:, kt, :],
                                               wkT_r[:, kt, :])
            for kt in range(NKT):
                eng = nc.gpsimd if kt % 2 == 0 else nc.scalar
                eng.dma_start(wv_sb[:, kt, :], wvT_r[:, kt, :])
            nc.sync.dma_start(x_blocks[1][:], xT_r[:, :, ts(1, 512)])
            for sb in range(1, NQB):
                nc.scalar.dma_start(cos_sb[:, ts(sb, 512)],
                                    cos_d.ap()[:, ts(sb, 512)])
                nc.scalar.dma_start(sin_sb[:, ts(sb, 512)],
                                    sin_d.ap()[:, ts(sb, 512)])
            nc.scalar.dma_start(msk_sb[:],
                                msk_d.ap().rearrange("o p q -> p o q"))
            nc.scalar.dma_start(wo_sb[:], woT_r)

            for sb in range(NQB):
                # ---------------- Phase A: projections + RoPE --------------
                x_sb = x_blocks.pop(sb)
                if sb + 2 < NQB:
                    x_blocks[sb + 2] = xpool.tile([128, NKT, 512], F16,
                                                  tag="x", name=f"x{sb+2}")
                    nc.sync.dma_start(x_blocks[sb + 2][:],
                                      xT_r[:, :, ts(sb + 2, 512)])
                sbs = ts(sb, 512)
                for h in range(HG):
                    for (w_sb, out_t) in ((wq_sb, qT), (wk_sb, kT)):
                        ps = psA.tile([128, 512], F32, tag="psA")
                        for kt in range(NKT):
                            nc.tensor.matmul(ps[:], w_sb[:, kt, ts(h, 128)],
                                             x_sb[:, kt, :],
                                             start=(kt == 0),
                                             stop=(kt == NKT - 1))
                        # RoPE: out = ps*cos + rot_half(ps)*sin
                        tmp = tmp_pool.tile([128, 512], F16, tag="rtmp")
                        nc.vector.tensor_mul(tmp[0:64, :], ps[64:128, :],
                                             sin_sb[0:64, sbs])
                        nc.vector.tensor_mul(tmp[64:128, :], ps[0:64, :],
                                             sin_sb[64:128, sbs])
                        dst = out_t[:, h, sbs]
                        nc.vector.tensor_mul(dst, ps[:], cos_sb[:, sbs])
                        nc.vector.tensor_add(dst, dst, tmp[:])
                for st in range(4):
                    ps = psA.tile([128, 512], F32, tag="psA")
                    for kt in range(NKT):
                        nc.tensor.matmul(ps[:], x_sb[:, kt, ts(st, 128)],
                                         wv_sb[:, kt, :],
                                         start=(kt == 0),
                                         stop=(kt == NKT - 1))
                    nc.vector.tensor_copy(v_sb[:, 4 * sb + st, :], ps[:])

                # ---------------- Phase B: attention for q-block sb --------
                qb = sb
                nkt = 4 * qb + 4
                for h in range(HG):
                    p_att = psT.tile([128, 512], F32, tag="psT")
                    p_den = psD.tile([1, 512], F32, tag="psD")
                    pts = {}

                    def drain(kt, last):
                        pt = pts.pop(kt)
                        nc.tensor.matmul(p_att[:], v_sb[:, kt, ts(h, 128)],
                                         pt[:],
                                         start=(kt == 0), stop=last)
                        nc.tensor.matmul(p_den[:], ones[:], pt[:],
                                         start=(kt == 0), stop=last)

                    for kt in range(nkt):
                        p_s = psS.tile([128, 512], F32, tag="psS")
                        diag = kt >= 4 * qb
                        nc.tensor.matmul(p_s[:], kT[:, h, ts(kt, 128)],
                                         qT[:, h, ts(qb, 512)],
                                         start=True, stop=not diag)
                        if diag:
                            # scores += I.T @ negmask (exp -> exact 0)
                            nc.tensor.matmul(p_s[:], ident[:],
                                             msk_sb[:, kt - 4 * qb, :],
                                             start=False, stop=True)
                        if kt >= 2:
                            drain(kt - 2, last=False)
                        pt = pt_pool.tile([128, 512], F16, tag="pt")
                        nc.scalar.activation(pt[:], p_s[:],
                                             mybir.ActivationFunctionType.Exp,
                                             scale=1.0 / math.sqrt(DH))
                        pts[kt] = pt
                    drain(nkt - 2, last=False)
                    drain(nkt - 1, last=True)
                    recip = nrm.tile([1, 512], F32, tag="recip")
                    nc.vector.reciprocal_approx_fast(recip[:], p_den[:])
                    rb = nrm.tile([128, 512], F32, tag="rb")
                    nc.gpsimd.partition_broadcast(rb[:], recip[:])
                    nc.vector.tensor_mul(attn_sb[:, h, ts(qb, 512)],
                                         p_att[:], rb[:])

                # ---------------- Phase C: output projection ---------------
                for qt in range(4 * qb, 4 * qb + 4):
                    for db in range(NQB):
                        py = psA.tile([128, 512], F32, tag="psA")
                        for ft in range(G):
                            nc.tensor.matmul(py[:],
                                             attn_sb[:, ft, ts(qt, 128)],
                                             wo_sb[:, ft, ts(db, 512)],
                                             start=(ft == 0),
                                             stop=(ft == G - 1))
                        y_sb = ystage.tile([128, 512], F16, tag="ysb")
                        if db % 2 == 0:
                            nc.scalar.copy(y_sb[:], py[:])
                        else:
                            nc.vector.tensor_copy(y_sb[:], py[:])
                        nc.sync.dma_start(y.ap()[ts(qt, 128), ts(db, 512)],
                                          y_sb[:])

    nc.compile()
    _cache[key] = nc
    return nc


def _in_maps(hidden_q, Wq, Wk, Wv, Wo):
    xs = hidden_q.astype(np.float32) / math.sqrt(D)
    xT = [np.ascontiguousarray(xs[b].T).astype(np.float16) for b in range(B)]
    cos_t, sin_t = _rope_tables()
    masks = _mask_tiles()
    wo_s = Wo.astype(np.float32) / math.sqrt(H * DH)
    in_maps = []
    for c in range(8):
        b, g = c // G, c % G
        rows = slice(F * g, F * (g + 1))
        in_maps.append({
            "xT": xT[b],
            "wqT": np.ascontiguousarray(Wq[rows, :].T).astype(np.float16),
            "wkT": np.ascontiguousarray(Wk[rows, :].T).astype(np.float16),
            "wvT": np.ascontiguousarray(Wv[rows, :].T).astype(np.float16),
            "woT": np.ascontiguousarray(wo_s[:, rows].T).astype(np.float16),
            "cos": cos_t, "sin": sin_t, "masks": masks,
        })
    return in_maps


def kernel(hidden_q, attention_mask, position_bias, Wq, Wk, Wv, Wo):
    hidden_q = np.asarray(hidden_q)
    Wq, Wk, Wv, Wo = (np.asarray(w) for w in (Wq, Wk, Wv, Wo))
    assert hidden_q.shape == (B, S, D)
    in_maps = _in_maps(hidden_q, Wq, Wk, Wv, Wo)
    nc = _build()
    res = run_bass_kernel_spmd(nc, in_maps, core_ids=list(range(8)))
    _cache["last_results"] = res
    out = np.zeros((B, S, D), np.float32)
    for c in range(8):
        out[c // G] += res.results[c]["y"]
    return out


# revision 24
# speedup vs baseline: 1.2075x; 1.0278x over previous
"""Multi-head causal self-attention with RoPE on 8 Trainium2 NeuronCores.

Reference computation (B=2, S=2048, D=2048, H=16, DH=128):
    xs = hidden_q / sqrt(D)
    q,k,v = xs @ {Wq,Wk,Wv}.T        (reshaped to [B,H,S,DH])
    q,k <- RoPE(q,k)
    scores = q @ k.T / sqrt(DH)  (causal masked)
    p = softmax(scores); attn = p @ v
    out = (attn / sqrt(H*DH)) @ Wo.T

Sharding: 8 cores = 2 (batch) x 4 (head-groups of 4 heads).  Each core
computes its head-group's projections, attention and a partial output
projection; the host sums the 4 partials per batch.

v6 design (all matmul operands fp16, PSUM fp32):
  * Q^T/K^T produced directly in [dh, seq] layout (weights stationary,
    x^T moving): no PE transposes, no DRAM spills.  RoPE uses a signed
    sin table (rows 0-63 hold -sin): 4 cross-partition DVE ops.
  * Causal mask added to scores in PSUM via identity.T @ (-30000 band);
    exp underflows to exact 0 -- the DVE stays out of the softmax chain.
  * Softmax denominator accumulates in a [1,512] PSUM bank via a
    ones-vector matmul per key tile (PE slots are cheaper than DVE ops
    here: DVE [128,512] ~800ns vs PE N=512 ~216ns).
  * Attention/denominator matmuls trail the score/exp pipeline by TWO
    key tiles so no PE instruction waits on a fresh exp semaphore
    (queue-head waits block the LDWEIGHTS pull-ahead, costing ~95ns on
    each following matmul).
  * attn output overwrites qT in place (the q slice of a (h,qb) is dead
    once its scores are done).
  * V-projection copies go through the Vector engine and y staging
    through Scalar, keeping the ACT queue clear of work that could
    delay phase-B exps.
  * DMA: wq/x0 split per contraction tile (gpsimd/sync queues), wk on
    the scalar queue, so the first projection chains chase arrivals.
    y partials are fp16; host sums 4 partials per batch in fp32.
"""

import math
from contextlib import ExitStack

import numpy as np

import concourse.bass as bass
import concourse.mybir as mybir
import concourse.tile as tile
from concourse import bacc
from concourse.bass import ts
from concourse.bass_utils import run_bass_kernel_spmd
from concourse.masks import make_identity

B, S, D, H, DH = 2, 2048, 2048, 16, 128
BASE = 10000.0
G = 4              # head-groups (cores per batch)
HG = H // G        # heads per group = 4
F = HG * DH        # features per group = 512
NT = S // 128      # 16 token tiles
NKT = D // 128     # 16 contraction tiles
NQB = S // 512     # 4 query blocks
NEG = -30000.0     # causal-mask bias; exp((s+NEG)/sqrt(DH)) == 0
F32 = mybir.dt.float32
F16 = mybir.dt.float16

_cache = {}


def _rope_tables():
    # [dh=128, S]; cos duplicated halves; sin rows 0-63 carry -sin
    inv_freq = 1.0 / (BASE ** (np.arange(0, DH, 2, dtype=np.float64) / DH))
    t = np.arange(S, dtype=np.float64)
    freqs = np.outer(inv_freq, t)                       # [64, S]
    cosT = np.concatenate([np.cos(freqs), np.cos(freqs)], 0)
    sinT = np.concatenate([-np.sin(freqs), np.sin(freqs)], 0)
    return cosT.astype(np.float16), sinT.astype(np.float16)


def _mask_tiles():
    # negmask[o][j, q] = 0 where key j+128*o <= query q, else NEG
    o = np.arange(4)[:, None, None]
    j = np.arange(128)[None, :, None]
    q = np.arange(512)[None, None, :]
    return np.where(q >= j + 128 * o, 0.0, NEG).astype(np.float16)


def _build(reps=1):
    key = ("nc", reps)
    if key in _cache:
        return _cache[key]
    nc = bacc.Bacc("TRN2", target_bir_lowering=False, debug=False, num_devices=8)

    xT = nc.dram_tensor("xT", [D, S], F16, kind="ExternalInput")
    wqT = nc.dram_tensor("wqT", [D, F], F16, kind="ExternalInput")
    wkT = nc.dram_tensor("wkT", [D, F], F16, kind="ExternalInput")
    wvT = nc.dram_tensor("wvT", [D, F], F16, kind="ExternalInput")
    woT = nc.dram_tensor("woT", [F, D], F16, kind="ExternalInput")
    cos_d = nc.dram_tensor("cos", [128, S], F16, kind="ExternalInput")
    sin_d = nc.dram_tensor("sin", [128, S], F16, kind="ExternalInput")
    msk_d = nc.dram_tensor("masks", [4, 128, 512], F16, kind="ExternalInput")
    y = nc.dram_tensor("y", [S, D], F16, kind="ExternalOutput")

    xT_r = xT.ap().rearrange("(kt p) s -> p kt s", p=128)       # [128, 16, S]
    wqT_r = wqT.ap().rearrange("(kt p) f -> p kt f", p=128)
    wkT_r = wkT.ap().rearrange("(kt p) f -> p kt f", p=128)
    wvT_r = wvT.ap().rearrange("(kt p) f -> p kt f", p=128)
    woT_r = woT.ap().rearrange("(ft p) d -> p ft d", p=128)

    with tile.TileContext(nc) as tc, ExitStack() as ctx:
        const = ctx.enter_context(tc.tile_pool(name="const", bufs=1))
        wpool = ctx.enter_context(tc.tile_pool(name="wpool", bufs=1))
        xpool = ctx.enter_context(tc.tile_pool(name="xpool", bufs=2))
        big = ctx.enter_context(tc.tile_pool(name="big", bufs=1))
        pt_pool = ctx.enter_context(tc.tile_pool(name="pt", bufs=18))
        tmp_pool = ctx.enter_context(tc.tile_pool(name="tmp", bufs=2))
        nrm = ctx.enter_context(tc.tile_pool(name="nrm", bufs=2))
        ystage = ctx.enter_context(tc.tile_pool(name="ystage", bufs=4))
        # PSUM: 2 + 3 + 2 + 1 banks = 8
        psA = ctx.enter_context(tc.tile_pool(name="psA", bufs=2, space="PSUM"))
        psS = ctx.enter_context(tc.tile_pool(name="psS", bufs=3, space="PSUM"))
        psT = ctx.enter_context(tc.tile_pool(name="psT", bufs=2, space="PSUM"))
        psD = ctx.enter_context(tc.tile_pool(name="psD", bufs=1, space="PSUM"))

        ones = const.tile([128, 1], F16, tag="ones")
        nc.gpsimd.memset(ones[:], 1.0)
        ident = const.tile([128, 128], F16, tag="ident")
        make_identity(nc, ident[:])
        msk_sb = const.tile([128, 4, 512], F16, tag="masks")

        # static loads; first chains chase per-kt arrivals
        wq_sb = wpool.tile([128, NKT, F], F16, tag="wq")
        wk_sb = wpool.tile([128, NKT, F], F16, tag="wk")
        wv_sb = wpool.tile([128, NKT, F], F16, tag="wv")
        wo_sb = wpool.tile([128, G, D], F16, tag="wo")
        cos_sb = wpool.tile([128, S], F16, tag="cos")
        sin_sb = wpool.tile([128, S], F16, tag="sin")
        for _rep in range(reps):
            qT = big.tile([128, HG, S], F16, tag="qT", name="qT")
            kT = big.tile([128, HG, S], F16, tag="kT", name="kT")
            v_sb = big.tile([128, NT, F], F16, tag="v", name="v")
            attn_sb = big.tile([128, HG, S], F16, tag="attn", name="attn")

            x_blocks = {}
            for sb in range(2):
                x_blocks[sb] = xpool.tile([128, NKT, 512], F16, tag="x",
                                          name=f"x{sb}")

            # First-needed slices up front, then the critical 6MB
            # (wq/wk/x-block-0) round-robined across all three DMA queues so
            # the first Q/K chains chase tile arrivals at aggregate
            # bandwidth; late-needed tensors (wv, tables, masks, wo) after.
            nc.scalar.dma_start(cos_sb[:, ts(0, 512)],
                                cos_d.ap()[:, ts(0, 512)])
            nc.scalar.dma_start(sin_sb[:, ts(0, 512)],
                                sin_d.ap()[:, ts(0, 512)])
            queues = (nc.gpsimd, nc.sync, nc.scalar)
            for kt in range(NKT):
                queues[kt % 3].dma_start(wq_sb[:, kt, :], wqT_r[:, kt, :])
                queues[(kt + 1) % 3].dma_start(x_blocks[0][:, kt, :],
                                               xT_r[:, kt, ts(0, 512)])
                queues[(kt + 2) % 3].dma_start(wk_sb[:, kt, :],
                                               wkT_r[:, kt, :])
            for kt in range(NKT):
                eng = nc.gpsimd if kt % 2 == 0 else nc.scalar
                eng.dma_start(wv_sb[:, kt, :], wvT_r[:, kt, :])
            nc.sync.dma_start(x_blocks[1][:], xT_r[:, :, ts(1, 512)])
            for sb in range(1, NQB):
                nc.scalar.dma_start(cos_sb[:, ts(sb, 512)],
                                    cos_d.ap()[:, ts(sb, 512)])
                nc.scalar.dma_start(sin_sb[:, ts(sb, 512)],
                                    sin_d.ap()[:, ts(sb, 512)])
            nc.scalar.dma_start(msk_sb[:],
                                msk_d.ap().rearrange("o p q -> p o q"))
            nc.scalar.dma_start(wo_sb[:], woT_r)

            for sb in range(NQB):
                # ---------------- Phase A: projections + RoPE --------------
                x_sb = x_blocks.pop(sb)
                if sb + 2 < NQB:
                    x_blocks[sb + 2] = xpool.tile([128, NKT, 512], F16,
                                                  tag="x", name=f"x{sb+2}")
                    nc.sync.dma_start(x_blocks[sb + 2][:],
                                      xT_r[:, :, ts(sb + 2, 512)])
                sbs = ts(sb, 512)
                for h in range(HG):
                    for (w_sb, out_t) in ((wq_sb, qT), (wk_sb, kT)):
                        ps = psA.tile([128, 512], F32, tag="psA")
                        for kt in range(NKT):
                            nc.tensor.matmul(ps[:], w_sb[:, kt, ts(h, 128)],
                                             x_sb[:, kt, :],
                                             start=(kt == 0),
                                             stop=(kt == NKT - 1))
                        # RoPE: out = ps*cos + rot_half(ps)*sin
                        tmp = tmp_pool.tile([128, 512], F16, tag="rtmp")
                        nc.vector.tensor_mul(tmp[0:64, :], ps[64:128, :],
                                             sin_sb[0:64, sbs])
                        nc.vector.tensor_mul(tmp[64:128, :], ps[0:64, :],
                                             sin_sb[64:128, sbs])
                        dst = out_t[:, h, sbs]
                        nc.vector.tensor_mul(dst, ps[:], cos_sb[:, sbs])
                        nc.vector.tensor_add(dst, dst, tmp[:])
                for st in range(4):
                    ps = psA.tile([128, 512], F32, tag="psA")
                    for kt in range(NKT):
                        nc.tensor.matmul(ps[:], x_sb[:, kt, ts(st, 128)],
                                         wv_sb[:, kt, :],
                                         start=(kt == 0),
                                         stop=(kt == NKT - 1))
                    nc.vector.tensor_copy(v_sb[:, 4 * sb + st, :], ps[:])

                # ---------------- Phase B: attention for q-block sb --------
                qb = sb
                nkt = 4 * qb + 4
                for h in range(HG):
                    p_att = psT.tile([128, 512], F32, tag="psT")
                    p_den = psD.tile([1, 512], F32, tag="psD")
                    pts = {}

                    def drain(kt, last):
                        nc.tensor.matmul(p_att[:], v_sb[:, kt, ts(h, 128)],
                                         pts[kt][:],
                                         start=(kt == 0), stop=last)

                    for kt in range(nkt):
                        p_s = psS.tile([128, 512], F32, tag="psS")
                        diag = kt >= 4 * qb
                        nc.tensor.matmul(p_s[:], kT[:, h, ts(kt, 128)],
                                         qT[:, h, ts(qb, 512)],
                                         start=True, stop=not diag)
                        if diag:
                            # scores += I.T @ negmask (exp -> exact 0)
                            nc.tensor.matmul(p_s[:], ident[:],
                                             msk_sb[:, kt - 4 * qb, :],
                                             start=False, stop=True)
                        if kt >= 2:
                            drain(kt - 2, last=False)
                        pt = pt_pool.tile([128, 512], F16, tag="pt")
                        nc.scalar.activation(pt[:], p_s[:],
                                             mybir.ActivationFunctionType.Exp,
                                             scale=1.0 / math.sqrt(DH))
                        pts[kt] = pt
                    drain(nkt - 2, last=False)
                    drain(nkt - 1, last=True)
                    # denominator burst: every pt is ready, so these run at
                    # full back-to-back rate with no semaphore waits, while
                    # ACT moves ahead into the next head's exps
                    for kt in range(nkt):
                        nc.tensor.matmul(p_den[:], ones[:], pts[kt][:],
                                         start=(kt == 0),
                                         stop=(kt == nkt - 1))
                    pts.clear()
                    recip = nrm.tile([1, 512], F32, tag="recip")
                    nc.vector.reciprocal_approx_fast(recip[:], p_den[:])
                    rb = nrm.tile([128, 512], F32, tag="rb")
                    nc.gpsimd.partition_broadcast(rb[:], recip[:])
                    nc.vector.tensor_mul(attn_sb[:, h, ts(qb, 512)],
                                         p_att[:], rb[:])

                # ---------------- Phase C: output projection ---------------
                for qt in range(4 * qb, 4 * qb + 4):
                    for db in range(NQB):
                        py = psA.tile([128, 512], F32, tag="psA")
                        for ft in range(G):
                            nc.tensor.matmul(py[:],
                                             attn_sb[:, ft, ts(qt, 128)],
                                             wo_sb[:, ft, ts(db, 512)],
                                             start=(ft == 0),
                                             stop=(ft == G - 1))
                        y_sb = ystage.tile([128, 512], F16, tag="ysb")
                        if db % 2 == 0:
                            nc.scalar.copy(y_sb[:], py[:])
                        else:
                            nc.vector.tensor_copy(y_sb[:], py[:])
                        nc.sync.dma_start(y.ap()[ts(qt, 128), ts(db, 512)],
                                          y_sb[:])

    nc.compile()
    _cache[key] = nc
    return nc


def _in_maps(hidden_q, Wq, Wk, Wv, Wo):
    xs = hidden_q.astype(np.float32) / math.sqrt(D)
    xT = [np.ascontiguousarray(xs[b].T).astype(np.float16) for b in range(B)]
    cos_t, sin_t = _rope_tables()
    masks = _mask_tiles()
    wo_s = Wo.astype(np.float32) / math.sqrt(H * DH)
    in_maps = []
    for c in range(8):
        b, g = c // G, c % G
        rows = slice(F * g, F * (g + 1))
        in_maps.append({
            "xT": xT[b],
            "wqT": np.ascontiguousarray(Wq[rows, :].T).astype(np.float16),
            "wkT": np.ascontiguousarray(Wk[rows, :].T).astype(np.float16),
            "wvT": np.ascontiguousarray(Wv[rows, :].T).astype(np.float16),
            "woT": np.ascontiguousarray(wo_s[:, rows].T).astype(np.float16),
            "cos": cos_t, "sin": sin_t, "masks": masks,
        })
    return in_maps


def kernel(hidden_q, attention_mask, position_bias, Wq, Wk, Wv, Wo):
    hidden_q = np.asarray(hidden_q)
    Wq, Wk, Wv, Wo = (np.asarray(w) for w in (Wq, Wk, Wv, Wo))
    assert hidden_q.shape == (B, S, D)
    in_maps = _in_maps(hidden_q, Wq, Wk, Wv, Wo)
    nc = _build()
    res = run_bass_kernel_spmd(nc, in_maps, core_ids=list(range(8)))
    _cache["last_results"] = res
    out = np.zeros((B, S, D), np.float32)
    for c in range(8):
        out[c // G] += res.results[c]["y"]
    return out


# revision 25
# speedup vs baseline: 1.2367x; 1.0242x over previous
"""Multi-head causal self-attention with RoPE on 8 Trainium2 NeuronCores.

Reference computation (B=2, S=2048, D=2048, H=16, DH=128):
    xs = hidden_q / sqrt(D)
    q,k,v = xs @ {Wq,Wk,Wv}.T        (reshaped to [B,H,S,DH])
    q,k <- RoPE(q,k)
    scores = q @ k.T / sqrt(DH)  (causal masked)
    p = softmax(scores); attn = p @ v
    out = (attn / sqrt(H*DH)) @ Wo.T

Sharding: 8 cores = 2 (batch) x 4 (head-groups of 4 heads).  Each core
computes its head-group's projections, attention and a partial output
projection; the host sums the 4 partials per batch.

v6 design (all matmul operands fp16, PSUM fp32):
  * Q^T/K^T produced directly in [dh, seq] layout (weights stationary,
    x^T moving): no PE transposes, no DRAM spills.  RoPE uses a signed
    sin table (rows 0-63 hold -sin): 4 cross-partition DVE ops.
  * Causal mask added to scores in PSUM via identity.T @ (-30000 band);
    exp underflows to exact 0 -- the DVE stays out of the softmax chain.
  * Softmax denominator accumulates in a [1,512] PSUM bank via a
    ones-vector matmul per key tile (PE slots are cheaper than DVE ops
    here: DVE [128,512] ~800ns vs PE N=512 ~216ns).
  * Attention/denominator matmuls trail the score/exp pipeline by TWO
    key tiles so no PE instruction waits on a fresh exp semaphore
    (queue-head waits block the LDWEIGHTS pull-ahead, costing ~95ns on
    each following matmul).
  * attn output overwrites qT in place (the q slice of a (h,qb) is dead
    once its scores are done).
  * V-projection copies go through the Vector engine and y staging
    through Scalar, keeping the ACT queue clear of work that could
    delay phase-B exps.
  * DMA: wq/x0 split per contraction tile (gpsimd/sync queues), wk on
    the scalar queue, so the first projection chains chase arrivals.
    y partials are fp16; host sums 4 partials per batch in fp32.
"""

import math
from contextlib import ExitStack

import numpy as np

import concourse.bass as bass
import concourse.mybir as mybir
import concourse.tile as tile
from concourse import bacc
from concourse.bass import ts
from concourse.bass_utils import run_bass_kernel_spmd
from concourse.masks import make_identity

B, S, D, H, DH = 2, 2048, 2048, 16, 128
BASE = 10000.0
G = 4              # head-groups (cores per batch)
HG = H // G        # heads per group = 4
F = HG * DH        # features per group = 512
NT = S // 128      # 16 token tiles
NKT = D // 128     # 16 contraction tiles
NQB = S // 512     # 4 query blocks
NEG = -30000.0     # causal-mask bias; exp((s+NEG)/sqrt(DH)) == 0
F32 = mybir.dt.float32
F16 = mybir.dt.float16

_cache = {}


def _rope_tables():
    # [dh=128, S]; cos duplicated halves; sin rows 0-63 carry -sin
    inv_freq = 1.0 / (BASE ** (np.arange(0, DH, 2, dtype=np.float64) / DH))
    t = np.arange(S, dtype=np.float64)
    freqs = np.outer(inv_freq, t)                       # [64, S]
    cosT = np.concatenate([np.cos(freqs), np.cos(freqs)], 0)
    sinT = np.concatenate([-np.sin(freqs), np.sin(freqs)], 0)
    return cosT.astype(np.float16), sinT.astype(np.float16)


def _mask_tiles():
    # negmask[o][j, q] = 0 where key j+128*o <= query q, else NEG
    o = np.arange(4)[:, None, None]
    j = np.arange(128)[None, :, None]
    q = np.arange(512)[None, None, :]
    return np.where(q >= j + 128 * o, 0.0, NEG).astype(np.float16)


def _build(reps=1):
    key = ("nc", reps)
    if key in _cache:
        return _cache[key]
    nc = bacc.Bacc("TRN2", target_bir_lowering=False, debug=False, num_devices=8)

    xT = nc.dram_tensor("xT", [D, S], F16, kind="ExternalInput")
    wqT = nc.dram_tensor("wqT", [D, F], F16, kind="ExternalInput")
    wkT = nc.dram_tensor("wkT", [D, F], F16, kind="ExternalInput")
    wvT = nc.dram_tensor("wvT", [D, F], F16, kind="ExternalInput")
    woT = nc.dram_tensor("woT", [F, D], F16, kind="ExternalInput")
    cos_d = nc.dram_tensor("cos", [128, S], F16, kind="ExternalInput")
    sin_d = nc.dram_tensor("sin", [128, S], F16, kind="ExternalInput")
    msk_d = nc.dram_tensor("masks", [4, 128, 512], F16, kind="ExternalInput")
    y = nc.dram_tensor("y", [S, D], F16, kind="ExternalOutput")

    xT_r = xT.ap().rearrange("(kt p) s -> p kt s", p=128)       # [128, 16, S]
    wqT_r = wqT.ap().rearrange("(kt p) f -> p kt f", p=128)
    wkT_r = wkT.ap().rearrange("(kt p) f -> p kt f", p=128)
    wvT_r = wvT.ap().rearrange("(kt p) f -> p kt f", p=128)
    woT_r = woT.ap().rearrange("(ft p) d -> p ft d", p=128)

    with tile.TileContext(nc) as tc, ExitStack() as ctx:
        const = ctx.enter_context(tc.tile_pool(name="const", bufs=1))
        wpool = ctx.enter_context(tc.tile_pool(name="wpool", bufs=1))
        xpool = ctx.enter_context(tc.tile_pool(name="xpool", bufs=2))
        big = ctx.enter_context(tc.tile_pool(name="big", bufs=1))
        pt_pool = ctx.enter_context(tc.tile_pool(name="pt", bufs=18))
        tmp_pool = ctx.enter_context(tc.tile_pool(name="tmp", bufs=2))
        nrm = ctx.enter_context(tc.tile_pool(name="nrm", bufs=2))
        ystage = ctx.enter_context(tc.tile_pool(name="ystage", bufs=4))
        # PSUM: 2 + 3 + 2 + 1 banks = 8
        psA = ctx.enter_context(tc.tile_pool(name="psA", bufs=2, space="PSUM"))
        psS = ctx.enter_context(tc.tile_pool(name="psS", bufs=3, space="PSUM"))
        psT = ctx.enter_context(tc.tile_pool(name="psT", bufs=2, space="PSUM"))
        psD = ctx.enter_context(tc.tile_pool(name="psD", bufs=1, space="PSUM"))

        ones = const.tile([128, 1], F16, tag="ones")
        nc.gpsimd.memset(ones[:], 1.0)
        ident = const.tile([128, 128], F16, tag="ident")
        make_identity(nc, ident[:])
        msk_sb = const.tile([128, 4, 512], F16, tag="masks")

        # static loads; first chains chase per-kt arrivals
        wq_sb = wpool.tile([128, NKT, F], F16, tag="wq")
        wk_sb = wpool.tile([128, NKT, F], F16, tag="wk")
        wv_sb = wpool.tile([128, NKT, F], F16, tag="wv")
        wo_sb = wpool.tile([128, G, D], F16, tag="wo")
        cos_sb = wpool.tile([128, S], F16, tag="cos")
        sin_sb = wpool.tile([128, S], F16, tag="sin")
        for _rep in range(reps):
            qT = big.tile([128, HG, S], F16, tag="qT", name="qT")
            kT = big.tile([128, HG, S], F16, tag="kT", name="kT")
            v_sb = big.tile([128, NT, F], F16, tag="v", name="v")
            attn_sb = big.tile([128, HG, S], F16, tag="attn", name="attn")

            x_blocks = {}
            for sb in range(2):
                x_blocks[sb] = xpool.tile([128, NKT, 512], F16, tag="x",
                                          name=f"x{sb}")

            # First-needed slices up front, then the critical 6MB
            # (wq/wk/x-block-0) round-robined across all three DMA queues so
            # the first Q/K chains chase tile arrivals at aggregate
            # bandwidth; late-needed tensors (wv, tables, masks, wo) after.
            nc.scalar.dma_start(cos_sb[:, ts(0, 512)],
                                cos_d.ap()[:, ts(0, 512)])
            nc.scalar.dma_start(sin_sb[:, ts(0, 512)],
                                sin_d.ap()[:, ts(0, 512)])
            queues = (nc.gpsimd, nc.sync, nc.scalar)
            for kt in range(NKT):
                queues[kt % 3].dma_start(wq_sb[:, kt, :], wqT_r[:, kt, :])
                queues[(kt + 1) % 3].dma_start(x_blocks[0][:, kt, :],
                                               xT_r[:, kt, ts(0, 512)])
                queues[(kt + 2) % 3].dma_start(wk_sb[:, kt, :],
                                               wkT_r[:, kt, :])
            for kt in range(NKT):
                eng = nc.gpsimd if kt % 2 == 0 else nc.scalar
                eng.dma_start(wv_sb[:, kt, :], wvT_r[:, kt, :])
            nc.sync.dma_start(x_blocks[1][:], xT_r[:, :, ts(1, 512)])
            for sb in range(1, NQB):
                nc.scalar.dma_start(cos_sb[:, ts(sb, 512)],
                                    cos_d.ap()[:, ts(sb, 512)])
                nc.scalar.dma_start(sin_sb[:, ts(sb, 512)],
                                    sin_d.ap()[:, ts(sb, 512)])
            nc.scalar.dma_start(msk_sb[:],
                                msk_d.ap().rearrange("o p q -> p o q"))
            nc.scalar.dma_start(wo_sb[:], woT_r)

            for sb in range(NQB):
                # ---------------- Phase A: projections + RoPE --------------
                x_sb = x_blocks.pop(sb)
                if sb + 2 < NQB:
                    x_blocks[sb + 2] = xpool.tile([128, NKT, 512], F16,
                                                  tag="x", name=f"x{sb+2}")
                    nc.sync.dma_start(x_blocks[sb + 2][:],
                                      xT_r[:, :, ts(sb + 2, 512)])
                sbs = ts(sb, 512)
                for h in range(HG):
                    for (w_sb, out_t) in ((wq_sb, qT), (wk_sb, kT)):
                        ps = psA.tile([128, 512], F32, tag="psA")
                        for kt in range(NKT):
                            nc.tensor.matmul(ps[:], w_sb[:, kt, ts(h, 128)],
                                             x_sb[:, kt, :],
                                             start=(kt == 0),
                                             stop=(kt == NKT - 1))
                        # RoPE: out = ps*cos + rot_half(ps)*sin
                        tmp = tmp_pool.tile([128, 512], F16, tag="rtmp")
                        nc.vector.tensor_mul(tmp[0:64, :], ps[64:128, :],
                                             sin_sb[0:64, sbs])
                        nc.vector.tensor_mul(tmp[64:128, :], ps[0:64, :],
                                             sin_sb[64:128, sbs])
                        dst = out_t[:, h, sbs]
                        nc.vector.tensor_mul(dst, ps[:], cos_sb[:, sbs])
                        nc.vector.tensor_add(dst, dst, tmp[:])
                for st in range(4):
                    ps = psA.tile([128, 512], F32, tag="psA")
                    for kt in range(NKT):
                        nc.tensor.matmul(ps[:], x_sb[:, kt, ts(st, 128)],
                                         wv_sb[:, kt, :],
                                         start=(kt == 0),
                                         stop=(kt == NKT - 1))
                    nc.vector.tensor_copy(v_sb[:, 4 * sb + st, :], ps[:])

                # ---------------- Phase B: attention for q-block sb --------
                qb = sb
                nkt = 4 * qb + 4
                for h in range(HG):
                    p_att = psT.tile([128, 512], F32, tag="psT")
                    p_den = psD.tile([1, 512], F32, tag="psD")
                    pts = {}

                    def drain(kt, last):
                        nc.tensor.matmul(p_att[:], v_sb[:, kt, ts(h, 128)],
                                         pts[kt][:],
                                         start=(kt == 0), stop=last)

                    for kt in range(nkt):
                        p_s = psS.tile([128, 512], F32, tag="psS")
                        diag = kt >= 4 * qb
                        nc.tensor.matmul(p_s[:], kT[:, h, ts(kt, 128)],
                                         qT[:, h, ts(qb, 512)],
                                         start=True, stop=not diag)
                        if diag:
                            # scores += I.T @ negmask (exp -> exact 0)
                            nc.tensor.matmul(p_s[:], ident[:],
                                             msk_sb[:, kt - 4 * qb, :],
                                             start=False, stop=True)
                        if kt >= 2:
                            drain(kt - 2, last=False)
                        pt = pt_pool.tile([128, 512], F16, tag="pt")
                        nc.scalar.activation(pt[:], p_s[:],
                                             mybir.ActivationFunctionType.Exp,
                                             scale=1.0 / math.sqrt(DH))
                        pts[kt] = pt
                    drain(nkt - 2, last=False)
                    drain(nkt - 1, last=True)
                    # denominator burst: every pt is ready, so these run at
                    # full back-to-back rate with no semaphore waits, while
                    # ACT moves ahead into the next head's exps
                    for kt in range(nkt):
                        nc.tensor.matmul(p_den[:], ones[:], pts[kt][:],
                                         start=(kt == 0),
                                         stop=(kt == nkt - 1))
                    pts.clear()
                    recip = nrm.tile([1, 512], F32, tag="recip")
                    nc.vector.reciprocal_approx_fast(recip[:], p_den[:])
                    rb = nrm.tile([128, 512], F32, tag="rb")
                    nc.gpsimd.partition_broadcast(rb[:], recip[:])
                    nc.vector.tensor_mul(attn_sb[:, h, ts(qb, 512)],
                                         p_att[:], rb[:])

                # ---------------- Phase C: output projection ---------------
                def emit_C(cqb):
                    for qt in range(4 * cqb, 4 * cqb + 4):
                        for db in range(NQB):
                            py = psA.tile([128, 512], F32, tag="psA")
                            for ft in range(G):
                                nc.tensor.matmul(py[:],
                                                 attn_sb[:, ft, ts(qt, 128)],
                                                 wo_sb[:, ft, ts(db, 512)],
                                                 start=(ft == 0),
                                                 stop=(ft == G - 1))
                            y_sb = ystage.tile([128, 512], F16, tag="ysb")
                            if db % 2 == 0:
                                nc.scalar.copy(y_sb[:], py[:])
                            else:
                                nc.vector.tensor_copy(y_sb[:], py[:])
                            nc.sync.dma_start(
                                y.ap()[ts(qt, 128), ts(db, 512)], y_sb[:])

                # C(2) is deferred until after B(3): phase A is finished by
                # then, so its matmuls are the only work available to fill
                # B(3)'s ACT-bound bubbles
                if qb < 2:
                    emit_C(qb)
                elif qb == 3:
                    emit_C(2)
                    emit_C(3)

    nc.compile()
    _cache[key] = nc
    return nc


def _in_maps(hidden_q, Wq, Wk, Wv, Wo):
    xs = hidden_q.astype(np.float32) / math.sqrt(D)
    xT = [np.ascontiguousarray(xs[b].T).astype(np.float16) for b in range(B)]
    cos_t, sin_t = _rope_tables()
    masks = _mask_tiles()
    wo_s = Wo.astype(np.float32) / math.sqrt(H * DH)
    in_maps = []
    for c in range(8):
        b, g = c // G, c % G
        rows = slice(F * g, F * (g + 1))
        in_maps.append({
            "xT": xT[b],
            "wqT": np.ascontiguousarray(Wq[rows, :].T).astype(np.float16),
            "wkT": np.ascontiguousarray(Wk[rows, :].T).astype(np.float16),
            "wvT": np.ascontiguousarray(Wv[rows, :].T).astype(np.float16),
            "woT": np.ascontiguousarray(wo_s[:, rows].T).astype(np.float16),
            "cos": cos_t, "sin": sin_t, "masks": masks,
        })
    return in_maps


def kernel(hidden_q, attention_mask, position_bias, Wq, Wk, Wv, Wo):
    hidden_q = np.asarray(hidden_q)
    Wq, Wk, Wv, Wo = (np.asarray(w) for w in (Wq, Wk, Wv, Wo))
    assert hidden_q.shape == (B, S, D)
    in_maps = _in_maps(hidden_q, Wq, Wk, Wv, Wo)
    nc = _build()
    res = run_bass_kernel_spmd(nc, in_maps, core_ids=list(range(8)))
    _cache["last_results"] = res
    out = np.zeros((B, S, D), np.float32)
    for c in range(8):
        out[c // G] += res.results[c]["y"]
    return out


# revision 26
# speedup vs baseline: 1.2543x; 1.0142x over previous
"""Multi-head causal self-attention with RoPE on 8 Trainium2 NeuronCores.

Reference computation (B=2, S=2048, D=2048, H=16, DH=128):
    xs = hidden_q / sqrt(D)
    q,k,v = xs @ {Wq,Wk,Wv}.T        (reshaped to [B,H,S,DH])
    q,k <- RoPE(q,k)
    scores = q @ k.T / sqrt(DH)  (causal masked)
    p = softmax(scores); attn = p @ v
    out = (attn / sqrt(H*DH)) @ Wo.T

Sharding: 8 cores = 2 (batch) x 4 (head-groups of 4 heads).  Each core
computes its head-group's projections, attention and a partial output
projection; the host sums the 4 partials per batch.

v6 design (all matmul operands fp16, PSUM fp32):
  * Q^T/K^T produced directly in [dh, seq] layout (weights stationary,
    x^T moving): no PE transposes, no DRAM spills.  RoPE uses a signed
    sin table (rows 0-63 hold -sin): 4 cross-partition DVE ops.
  * Causal mask added to scores in PSUM via identity.T @ (-30000 band);
    exp underflows to exact 0 -- the DVE stays out of the softmax chain.
  * Softmax denominator accumulates in a [1,512] PSUM bank via a
    ones-vector matmul per key tile (PE slots are cheaper than DVE ops
    here: DVE [128,512] ~800ns vs PE N=512 ~216ns).
  * Attention/denominator matmuls trail the score/exp pipeline by TWO
    key tiles so no PE instruction waits on a fresh exp semaphore
    (queue-head waits block the LDWEIGHTS pull-ahead, costing ~95ns on
    each following matmul).
  * attn output overwrites qT in place (the q slice of a (h,qb) is dead
    once its scores are done).
  * V-projection copies go through the Vector engine and y staging
    through Scalar, keeping the ACT queue clear of work that could
    delay phase-B exps.
  * DMA: wq/x0 split per contraction tile (gpsimd/sync queues), wk on
    the scalar queue, so the first projection chains chase arrivals.
    y partials are fp16; host sums 4 partials per batch in fp32.
"""

import math
from contextlib import ExitStack

import numpy as np

import concourse.bass as bass
import concourse.mybir as mybir
import concourse.tile as tile
from concourse import bacc
from concourse.bass import ts
from concourse.bass_utils import run_bass_kernel_spmd
from concourse.masks import make_identity

B, S, D, H, DH = 2, 2048, 2048, 16, 128
BASE = 10000.0
G = 4              # head-groups (cores per batch)
HG = H // G        # heads per group = 4
F = HG * DH        # features per group = 512
NT = S // 128      # 16 token tiles
NKT = D // 128     # 16 contraction tiles
NQB = S // 512     # 4 query blocks
NEG = -30000.0     # causal-mask bias; exp((s+NEG)/sqrt(DH)) == 0
F32 = mybir.dt.float32
F16 = mybir.dt.float16

_cache = {}


def _rope_tables():
    # [dh=128, S]; cos duplicated halves; sin rows 0-63 carry -sin
    inv_freq = 1.0 / (BASE ** (np.arange(0, DH, 2, dtype=np.float64) / DH))
    t = np.arange(S, dtype=np.float64)
    freqs = np.outer(inv_freq, t)                       # [64, S]
    cosT = np.concatenate([np.cos(freqs), np.cos(freqs)], 0)
    sinT = np.concatenate([-np.sin(freqs), np.sin(freqs)], 0)
    return cosT.astype(np.float16), sinT.astype(np.float16)


def _mask_tiles():
    # negmask[o][j, q] = 0 where key j+128*o <= query q, else NEG
    o = np.arange(4)[:, None, None]
    j = np.arange(128)[None, :, None]
    q = np.arange(512)[None, None, :]
    return np.where(q >= j + 128 * o, 0.0, NEG).astype(np.float16)


def _build(reps=1):
    key = ("nc", reps)
    if key in _cache:
        return _cache[key]
    nc = bacc.Bacc("TRN2", target_bir_lowering=False, debug=False, num_devices=8)

    xT = nc.dram_tensor("xT", [D, S], F16, kind="ExternalInput")
    wqT = nc.dram_tensor("wqT", [D, F], F16, kind="ExternalInput")
    wkT = nc.dram_tensor("wkT", [D, F], F16, kind="ExternalInput")
    wvT = nc.dram_tensor("wvT", [D, F], F16, kind="ExternalInput")
    woT = nc.dram_tensor("woT", [F, D], F16, kind="ExternalInput")
    cos_d = nc.dram_tensor("cos", [128, S], F16, kind="ExternalInput")
    sin_d = nc.dram_tensor("sin", [128, S], F16, kind="ExternalInput")
    msk_d = nc.dram_tensor("masks", [4, 128, 512], F16, kind="ExternalInput")
    y = nc.dram_tensor("y", [S, D], F16, kind="ExternalOutput")

    xT_r = xT.ap().rearrange("(kt p) s -> p kt s", p=128)       # [128, 16, S]
    wqT_r = wqT.ap().rearrange("(kt p) f -> p kt f", p=128)
    wkT_r = wkT.ap().rearrange("(kt p) f -> p kt f", p=128)
    wvT_r = wvT.ap().rearrange("(kt p) f -> p kt f", p=128)
    woT_r = woT.ap().rearrange("(ft p) d -> p ft d", p=128)

    with tile.TileContext(nc) as tc, ExitStack() as ctx:
        const = ctx.enter_context(tc.tile_pool(name="const", bufs=1))
        wpool = ctx.enter_context(tc.tile_pool(name="wpool", bufs=1))
        xpool = ctx.enter_context(tc.tile_pool(name="xpool", bufs=2))
        big = ctx.enter_context(tc.tile_pool(name="big", bufs=1))
        pt_pool = ctx.enter_context(tc.tile_pool(name="pt", bufs=18))
        tmp_pool = ctx.enter_context(tc.tile_pool(name="tmp", bufs=2))
        nrm = ctx.enter_context(tc.tile_pool(name="nrm", bufs=2))
        ystage = ctx.enter_context(tc.tile_pool(name="ystage", bufs=4))
        # PSUM: 2 + 3 + 2 + 1 banks = 8
        psA = ctx.enter_context(tc.tile_pool(name="psA", bufs=2, space="PSUM"))
        psS = ctx.enter_context(tc.tile_pool(name="psS", bufs=3, space="PSUM"))
        psT = ctx.enter_context(tc.tile_pool(name="psT", bufs=2, space="PSUM"))
        psD = ctx.enter_context(tc.tile_pool(name="psD", bufs=1, space="PSUM"))

        ones = const.tile([128, 1], F16, tag="ones")
        nc.gpsimd.memset(ones[:], 1.0)
        ident = const.tile([128, 128], F16, tag="ident")
        make_identity(nc, ident[:])
        msk_sb = const.tile([128, 4, 512], F16, tag="masks")

        # static loads; first chains chase per-kt arrivals
        wq_sb = wpool.tile([128, NKT, F], F16, tag="wq")
        wk_sb = wpool.tile([128, NKT, F], F16, tag="wk")
        wv_sb = wpool.tile([128, NKT, F], F16, tag="wv")
        wo_sb = wpool.tile([128, G, D], F16, tag="wo")
        cos_sb = wpool.tile([128, S], F16, tag="cos")
        sin_sb = wpool.tile([128, S], F16, tag="sin")
        for _rep in range(reps):
            qT = big.tile([128, HG, S], F16, tag="qT", name="qT")
            kT = big.tile([128, HG, S], F16, tag="kT", name="kT")
            v_sb = big.tile([128, NT, F], F16, tag="v", name="v")
            attn_sb = big.tile([128, HG, S], F16, tag="attn", name="attn")

            x_blocks = {}
            for sb in range(2):
                x_blocks[sb] = xpool.tile([128, NKT, 512], F16, tag="x",
                                          name=f"x{sb}")

            # First-needed slices up front, then the critical 6MB
            # (wq/wk/x-block-0) round-robined across all three DMA queues so
            # the first Q/K chains chase tile arrivals at aggregate
            # bandwidth; late-needed tensors (wv, tables, masks, wo) after.
            nc.scalar.dma_start(cos_sb[:, ts(0, 512)],
                                cos_d.ap()[:, ts(0, 512)])
            nc.scalar.dma_start(sin_sb[:, ts(0, 512)],
                                sin_d.ap()[:, ts(0, 512)])
            queues = (nc.gpsimd, nc.sync, nc.scalar)
            for kt in range(NKT):
                queues[kt % 3].dma_start(wq_sb[:, kt, :], wqT_r[:, kt, :])
                queues[(kt + 1) % 3].dma_start(x_blocks[0][:, kt, :],
                                               xT_r[:, kt, ts(0, 512)])
                queues[(kt + 2) % 3].dma_start(wk_sb[:, kt, :],
                                               wkT_r[:, kt, :])
            for kt in range(NKT):
                eng = nc.gpsimd if kt % 2 == 0 else nc.scalar
                eng.dma_start(wv_sb[:, kt, :], wvT_r[:, kt, :])
            nc.sync.dma_start(x_blocks[1][:], xT_r[:, :, ts(1, 512)])
            for sb in range(1, NQB):
                nc.scalar.dma_start(cos_sb[:, ts(sb, 512)],
                                    cos_d.ap()[:, ts(sb, 512)])
                nc.scalar.dma_start(sin_sb[:, ts(sb, 512)],
                                    sin_d.ap()[:, ts(sb, 512)])
            nc.scalar.dma_start(msk_sb[:],
                                msk_d.ap().rearrange("o p q -> p o q"))
            nc.scalar.dma_start(wo_sb[:], woT_r)

            for sb in range(NQB):
                # ---------------- Phase A: projections + RoPE --------------
                x_sb = x_blocks.pop(sb)
                if sb + 2 < NQB:
                    x_blocks[sb + 2] = xpool.tile([128, NKT, 512], F16,
                                                  tag="x", name=f"x{sb+2}")
                    nc.sync.dma_start(x_blocks[sb + 2][:],
                                      xT_r[:, :, ts(sb + 2, 512)])
                sbs = ts(sb, 512)
                for h in range(HG):
                    for (w_sb, out_t) in ((wq_sb, qT), (wk_sb, kT)):
                        ps = psA.tile([128, 512], F32, tag="psA")
                        for kt in range(NKT):
                            nc.tensor.matmul(ps[:], w_sb[:, kt, ts(h, 128)],
                                             x_sb[:, kt, :],
                                             start=(kt == 0),
                                             stop=(kt == NKT - 1))
                        # RoPE: out = ps*cos + rot_half(ps)*sin
                        tmp = tmp_pool.tile([128, 512], F16, tag="rtmp")
                        nc.vector.tensor_mul(tmp[0:64, :], ps[64:128, :],
                                             sin_sb[0:64, sbs])
                        nc.vector.tensor_mul(tmp[64:128, :], ps[0:64, :],
                                             sin_sb[64:128, sbs])
                        dst = out_t[:, h, sbs]
                        nc.vector.tensor_mul(dst, ps[:], cos_sb[:, sbs])
                        nc.vector.tensor_add(dst, dst, tmp[:])
                for st in range(4):
                    ps = psA.tile([128, 512], F32, tag="psA")
                    for kt in range(NKT):
                        nc.tensor.matmul(ps[:], x_sb[:, kt, ts(st, 128)],
                                         wv_sb[:, kt, :],
                                         start=(kt == 0),
                                         stop=(kt == NKT - 1))
                    nc.vector.tensor_copy(v_sb[:, 4 * sb + st, :], ps[:])

                # ---------------- Phase B: attention for q-block sb --------
                qb = sb
                nkt = 4 * qb + 4
                for h in range(HG):
                    p_att = psT.tile([128, 512], F32, tag="psT")
                    p_den = psD.tile([1, 512], F32, tag="psD")
                    pts = {}

                    def drain(kt, last):
                        nc.tensor.matmul(p_att[:], v_sb[:, kt, ts(h, 128)],
                                         pts[kt][:],
                                         start=(kt == 0), stop=last)

                    for kt in range(nkt):
                        p_s = psS.tile([128, 512], F32, tag="psS")
                        diag = kt >= 4 * qb
                        nc.tensor.matmul(p_s[:], kT[:, h, ts(kt, 128)],
                                         qT[:, h, ts(qb, 512)],
                                         start=True, stop=not diag)
                        if diag:
                            # scores += I.T @ negmask (exp -> exact 0)
                            nc.tensor.matmul(p_s[:], ident[:],
                                             msk_sb[:, kt - 4 * qb, :],
                                             start=False, stop=True)
                        if kt >= 2:
                            drain(kt - 2, last=False)
                        pt = pt_pool.tile([128, 512], F16, tag="pt")
                        nc.scalar.activation(pt[:], p_s[:],
                                             mybir.ActivationFunctionType.Exp,
                                             scale=1.0 / math.sqrt(DH))
                        pts[kt] = pt
                    drain(nkt - 2, last=False)
                    drain(nkt - 1, last=True)
                    # denominator burst: every pt is ready, so these run at
                    # full back-to-back rate with no semaphore waits, while
                    # ACT moves ahead into the next head's exps
                    for kt in range(nkt):
                        nc.tensor.matmul(p_den[:], ones[:], pts[kt][:],
                                         start=(kt == 0),
                                         stop=(kt == nkt - 1))
                    pts.clear()
                    recip = nrm.tile([1, 512], F32, tag="recip")
                    nc.vector.reciprocal_approx_fast(recip[:], p_den[:])
                    rb = nrm.tile([128, 512], F32, tag="rb")
                    nc.gpsimd.partition_broadcast(rb[:], recip[:])
                    nc.vector.tensor_mul(attn_sb[:, h, ts(qb, 512)],
                                         p_att[:], rb[:])

                # ---------------- Phase C: output projection ---------------
                def emit_C(cqb):
                    for qt in range(4 * cqb, 4 * cqb + 4):
                        for db in range(NQB):
                            py = psA.tile([128, 512], F32, tag="psA")
                            for ft in range(G):
                                nc.tensor.matmul(py[:],
                                                 attn_sb[:, ft, ts(qt, 128)],
                                                 wo_sb[:, ft, ts(db, 512)],
                                                 start=(ft == 0),
                                                 stop=(ft == G - 1))
                            y_sb = ystage.tile([128, 512], F16, tag="ysb")
                            if db % 2 == 0:
                                nc.scalar.copy(y_sb[:], py[:])
                            else:
                                nc.vector.tensor_copy(y_sb[:], py[:])
                            nc.sync.dma_start(
                                y.ap()[ts(qt, 128), ts(db, 512)], y_sb[:])

                # every C block is deferred one attention block (C(qb) after
                # B(qb+1)) so each ACT-bound B block has ready C matmuls --
                # in addition to A chains -- to fill its PE bubbles
                if qb >= 1:
                    emit_C(qb - 1)
                if qb == 3:
                    emit_C(3)

    nc.compile()
    _cache[key] = nc
    return nc


def _in_maps(hidden_q, Wq, Wk, Wv, Wo):
    xs = hidden_q.astype(np.float32) / math.sqrt(D)
    xT = [np.ascontiguousarray(xs[b].T).astype(np.float16) for b in range(B)]
    cos_t, sin_t = _rope_tables()
    masks = _mask_tiles()
    wo_s = Wo.astype(np.float32) / math.sqrt(H * DH)
    in_maps = []
    for c in range(8):
        b, g = c // G, c % G
        rows = slice(F * g, F * (g + 1))
        in_maps.append({
            "xT": xT[b],
            "wqT": np.ascontiguousarray(Wq[rows, :].T).astype(np.float16),
            "wkT": np.ascontiguousarray(Wk[rows, :].T).astype(np.float16),
            "wvT": np.ascontiguousarray(Wv[rows, :].T).astype(np.float16),
            "woT": np.ascontiguousarray(wo_s[:, rows].T).astype(np.float16),
            "cos": cos_t, "sin": sin_t, "masks": masks,
        })
    return in_maps


def kernel(hidden_q, attention_mask, position_bias, Wq, Wk, Wv, Wo):
    hidden_q = np.asarray(hidden_q)
    Wq, Wk, Wv, Wo = (np.asarray(w) for w in (Wq, Wk, Wv, Wo))
    assert hidden_q.shape == (B, S, D)
    in_maps = _in_maps(hidden_q, Wq, Wk, Wv, Wo)
    nc = _build()
    res = run_bass_kernel_spmd(nc, in_maps, core_ids=list(range(8)))
    _cache["last_results"] = res
    out = np.zeros((B, S, D), np.float32)
    for c in range(8):
        out[c // G] += res.results[c]["y"]
    return out
